# revision 33
# baseline (speedup 1.0000x reference)
"""TRN2 Bass kernel: 3x3 valid cross-correlation + bias on [8192, 8192] fp32.

Strategy (memory-regime, rel-err budget 2e-2):
- fp16 end-to-end on the wire: the host casts x to fp16, the device computes
  fp16 banded matmuls (1 cycle/row on the PE, same as bf16) accumulating in
  fp32 PSUM, and writes fp16 output that the host casts back to fp32. This
  halves HBM traffic (the roofline for this kernel) and keeps relative error
  ~4e-4, well inside the 2e-2 gate.
- Row sharding across the 8 cores (1026-row input shards incl. the 2-row
  halo; weight/bias replicated). Each core streams 9 stripes of 116 input
  rows (114 output rows) x 8192 cols, so every HBM transfer is a single
  contiguous ~1.9 MB DMA - big enough for ~97% DMA efficiency, unlike
  many-small-transfer layouts that hot-spot one SDMA engine.
- Per stripe the 3x3 conv is 3 PSUM-accumulated matmuls per 512-col chunk:
  the column (dy) taps become a 3-banded stationary matrix B_dx[k, m] =
  w[k-m, dx] and the row (dx) taps are free-dim shifts of the moving x tile.
  PSUM eviction (+bias, cast to fp16) alternates between the scalar and
  vector engines so neither becomes the critical path.
"""
import numpy as np
from contextlib import ExitStack

import concourse.bass as bass
import concourse.tile as tile
from concourse import mybir, bacc
from concourse.bass_utils import run_bass_kernel_spmd

H = W = 8192
KH = KW = 3
OH, OW = H - KH + 1, W - KW + 1           # 8190 x 8190
NCORES = 8
SHARD_OH = 1024                           # output rows per core
SHARD_IH = SHARD_OH + KH - 1              # 1026 input rows per core
NSTRIPES = 9
STRIPE_O = 114                            # output rows per stripe (8 stripes)
STRIPE_I = STRIPE_O + KH - 1              # 116 input rows per stripe
CHUNK = 512                               # matmul moving free dim (PSUM bank)

F32 = mybir.dt.float32
F16 = mybir.dt.float16
I32 = mybir.dt.int32
OP = mybir.AluOpType


def build_nc(sb_bufs=6, ps_bufs=8, dve_hwdge=False, n_swq=1):
    nc = bacc.Bacc("TRN2", target_bir_lowering=False, debug=False,
                   num_devices=NCORES, num_swdge_queues=n_swq)
    if dve_hwdge:
        # Register DVE as a third HWDGE engine: its qDVEDynamicHW ring gives
        # the kernel another independent hardware-DGE DMA stream.
        nc.hwdge_engines.add(mybir.EngineType.DVE)
        nc.m.queues.append(mybir.DMAQueue(
            type="dynamic", name="qDVEDynamicHW", blocks=[],
            engine=mybir.EngineType.DVE, location_alt=False,
            num_queues=16, is_HWDGE=True, num_semaphores=0, semaphores=[]))
    x_sh = nc.dram_tensor("x_sh", [SHARD_IH, W], F16, kind="ExternalInput").ap()
    w_d = nc.dram_tensor("w", [KH, KW], F32, kind="ExternalInput").ap()
    b_d = nc.dram_tensor("b", [1], F32, kind="ExternalInput").ap()
    out_sh = nc.dram_tensor("out_sh", [SHARD_OH, OW], F16,
                            kind="ExternalOutput").ap()

    with tile.TileContext(nc) as tc, ExitStack() as ctx:
        consts = ctx.enter_context(tc.tile_pool(name="consts", bufs=1))
        xin = ctx.enter_context(tc.tile_pool(name="xin", bufs=sb_bufs))
        outp = ctx.enter_context(tc.tile_pool(name="outp", bufs=sb_bufs))
        psum = ctx.enter_context(tc.tile_pool(name="psum", bufs=ps_bufs,
                                              space="PSUM"))

        # ---- one-time setup: broadcast weights, build fp16 band matrices ----
        # wb[:, j] = w[j//3, j%3] for all partitions; wb[:, 9] = bias
        wb = consts.tile([128, 10], F32)
        nc.sync.dma_start(wb[:, 0:9], w_d.rearrange("a b -> (a b)")
                          .unsqueeze(0).partition_broadcast(128))
        nc.sync.dma_start(wb[:, 9:10], b_d.unsqueeze(0).partition_broadcast(128))

        # diag[p, m] = p - m ; mask_dy = (diag == dy)
        diag = consts.tile([128, STRIPE_O], I32)
        nc.gpsimd.iota(diag[:], pattern=[[-1, STRIPE_O]], base=0,
                       channel_multiplier=1)
        masks = []
        for dy in range(KH):
            m = consts.tile([128, STRIPE_O], F32, tag=f"mask{dy}")
            nc.vector.tensor_scalar(m[:], diag[:], dy, None, OP.is_equal)
            masks.append(m)
        # band matrices: band16[dx][k, m] = w[k-m, dx] (fp16 stationary)
        bands = []
        bf = consts.tile([128, STRIPE_O], F32)
        for dx in range(KW):
            nc.vector.tensor_scalar(bf[:], masks[0][:], wb[:, dx:dx + 1],
                                    None, OP.mult)
            for dy in range(1, KH):
                j = 3 * dy + dx
                nc.vector.scalar_tensor_tensor(bf[:], masks[dy][:],
                                               wb[:, j:j + 1], bf[:],
                                               OP.mult, OP.add)
            b16 = consts.tile([128, STRIPE_O], F16, tag=f"band{dx}")
            nc.scalar.copy(b16[:], bf[:])
            bands.append(b16)

        # ---- main loop over row stripes ----
        # Every stripe transfer is split into 4 column blocks of 2048 cols
        # (4KB per-partition lines) spread across the three DMA queues:
        # HWDGE sync/scalar streams sustain ~54 GB/s each, SWDGE (gpsimd)
        # ~190 GB/s over all 16 engines. Rotating ~1.5 of 4 blocks onto the
        # HWDGE queue balances the three paths, and the block granularity
        # lets matmuls start as soon as the first block lands instead of
        # waiting out a full-stripe DMA.
        nchunks = (OW + CHUNK - 1) // CHUNK            # 16 (last chunk 510)
        BLK = 2048
        nblk = 4

        def in_blocks(s):
            # Inputs ride SWDGE mostly: they gate the matmuls, and SWDGE both
            # spreads over all 16 engines and keeps latency low (~3us/block).
            # Every other stripe hands one block to a HWDGE queue for balance;
            # the 5-stripe lookahead hides that stream's higher latency.
            r0 = s * STRIPE_O
            n_in = min(STRIPE_I, SHARD_IH - r0)
            xt = xin.tile([n_in, W], F16, name=f"xt{s}", tag="xt")
            for b in range(nblk):
                c0, c1 = b * BLK, (b + 1) * BLK
                gp_dma(xt[:, c0:c1], x_sh[r0:r0 + n_in, c0:c1])
            return xt

        hw_engines = ([nc.sync, nc.scalar, nc.vector] if dve_hwdge
                      else [nc.sync, nc.scalar])
        out_rr = [0]
        swq_rr = [0]

        def gp_dma(out_ap_, in_ap_):
            # Round-robin SWDGE transfers over the allocated qPoolDynamic{i}
            # queues. dma_start always targets queue 0, so the extra queues
            # are reached by emitting the InstDMACopy directly.
            q = swq_rr[0] % n_swq
            swq_rr[0] += 1
            eng = nc.gpsimd
            if q == 0:
                eng.dma_start(out_ap_, in_ap_)
                return
            o, i = bass.balance_dma_aps(out_ap_, in_ap_)
            oap = eng.lower_ap_dma(o, force_symbolic=False,
                                   has_bounds_check=False)
            iap = eng.lower_ap_dma(i, force_symbolic=False,
                                   has_bounds_check=False)
            eng.add_instruction(mybir.InstDMACopy(
                name=nc.get_next_instruction_name(),
                queue=f"qPoolDynamic{q}", mode="Copy",
                ins=[*iap], outs=[*oap], oob_is_err=True,
                cce_op=mybir.AluOpType.bypass,
                bass_cond_hint=None, single_packet=False))

        def out_blocks(s, ot, n_out):
            # Outputs only need to drain by the end: push ~5/8 of them onto
            # the HWDGE queues (round-robin), rest onto SWDGE. The final
            # stripe is shredded into 1024-col pieces over every queue so the
            # drain tail after the last matmul is as short as possible.
            r0 = s * STRIPE_O
            n_hw = 3 if s % 2 == 0 else 2
            hw = {(s + i) % nblk for i in range(n_hw)}
            for b in range(nblk):
                c0, c1 = b * BLK, min((b + 1) * BLK, OW)
                if b in hw:
                    eng = hw_engines[out_rr[0] % len(hw_engines)]
                    out_rr[0] += 1
                    eng.dma_start(out_sh[r0:r0 + n_out, c0:c1], ot[:, c0:c1])
                else:
                    gp_dma(out_sh[r0:r0 + n_out, c0:c1], ot[:, c0:c1])

        # Out-block emission runs TWO stripes behind compute: their eviction
        # waits are then already satisfied, so they never head-of-line-block
        # the gpsimd FIFO between successive stripes' input triggers.
        pending = []
        for s in range(NSTRIPES):
            r0 = s * STRIPE_O
            n_in = min(STRIPE_I, SHARD_IH - r0)        # 116 (114 for s=8)
            n_out = n_in - (KH - 1)
            xt = in_blocks(s)
            if len(pending) >= 2:
                out_blocks(*pending.pop(0))
            ot = outp.tile([n_out, OW], F16, name=f"ot{s}", tag="ot")
            for c in range(nchunks):
                n0 = c * CHUNK
                free = min(CHUNK, OW - n0)
                pt = psum.tile([n_out, CHUNK], F32)
                for dx in range(KW):
                    nc.tensor.matmul(pt[:, :free],
                                     bands[dx][:n_in, :n_out],
                                     xt[:, n0 + dx:n0 + dx + free],
                                     start=(dx == 0), stop=(dx == KW - 1))
                if c % 2 == 0:
                    nc.scalar.activation(
                        ot[:, n0:n0 + free], pt[:, :free],
                        mybir.ActivationFunctionType.Identity,
                        bias=wb[0:n_out, 9:10])
                else:
                    nc.vector.tensor_scalar(ot[:, n0:n0 + free], pt[:, :free],
                                            wb[0:n_out, 9:10], None, OP.add)
            pending.append((s, ot, n_out))
        while pending:
            out_blocks(*pending.pop(0))
    nc.compile()
    return nc


_nc_cache = {}


def _get_nc(**kw):
    key = tuple(sorted(kw.items()))
    if key not in _nc_cache:
        _nc_cache[key] = build_nc(**kw)
    return _nc_cache[key]


def shard_inputs(x, weight, bias):
    x16 = np.asarray(x, dtype=np.float16)
    weight = np.ascontiguousarray(np.asarray(weight, dtype=np.float32))
    bias = np.ascontiguousarray(np.asarray(bias, dtype=np.float32))
    row0 = [min(c * SHARD_OH, H - SHARD_IH) for c in range(NCORES)]
    in_maps = [{"x_sh": np.ascontiguousarray(x16[r0:r0 + SHARD_IH, :]),
                "w": weight, "b": bias} for r0 in row0]
    return in_maps, row0


def unshard_outputs(results, row0):
    out = np.empty((OH, OW), dtype=np.float32)
    for c in range(NCORES):
        sh = results[c]["out_sh"]
        lo = c * SHARD_OH
        hi = min(lo + SHARD_OH, OH)
        off = lo - row0[c]
        out[lo:hi, :] = sh[off:off + (hi - lo), :].astype(np.float32)
    return out


def kernel(x, weight, bias, **build_kw):
    nc = _get_nc(**build_kw)
    in_maps, row0 = shard_inputs(x, weight, bias)
    res = run_bass_kernel_spmd(nc, in_maps, list(range(NCORES)))
    return unshard_outputs(res.results, row0)


# revision 38
# speedup vs baseline: 1.0071x; 1.0071x over previous
"""TRN2 Bass kernel: 3x3 valid cross-correlation + bias on [8192, 8192] fp32.

Strategy (memory-regime, rel-err budget 2e-2):
- fp16 end-to-end on the wire: the host casts x to fp16, the device computes
  fp16 banded matmuls (1 cycle/row on the PE, same as bf16) accumulating in
  fp32 PSUM, and writes fp16 output that the host casts back to fp32. This
  halves HBM traffic (the roofline for this kernel) and keeps relative error
  ~4e-4, well inside the 2e-2 gate.
- Row sharding across the 8 cores (1026-row input shards incl. the 2-row
  halo; weight/bias replicated). Each core streams 9 stripes of 116 input
  rows (114 output rows) x 8192 cols. Stripe transfers are split into
  2048-col blocks and spread over the SWDGE queue (all 16 SDMA engines,
  ~150 GB/s) plus the two HWDGE queues (~54 GB/s each) to aggregate all
  available dynamic-DMA bandwidth (~195 GB/s effective).
- Per stripe the 3x3 conv is 3 PSUM-accumulated matmuls per 512-col chunk:
  the column (dy) taps become a 3-banded stationary matrix B_dx[k, m] =
  w[k-m, dx] and the row (dx) taps are free-dim shifts of the moving x tile.
  PSUM eviction (+bias, cast to fp16) alternates between the scalar and
  vector engines so neither becomes the critical path.
"""
import numpy as np
from contextlib import ExitStack

import concourse.bass as bass
import concourse.tile as tile
from concourse import mybir, bacc
from concourse.bass_utils import run_bass_kernel_spmd

H = W = 8192
KH = KW = 3
OH, OW = H - KH + 1, W - KW + 1           # 8190 x 8190
NCORES = 8
SHARD_OH = 1024                           # output rows per core
SHARD_IH = SHARD_OH + KH - 1              # 1026 input rows per core
NSTRIPES = 9
STRIPE_O = 114                            # output rows per stripe (8 stripes)
STRIPE_I = STRIPE_O + KH - 1              # 116 input rows per stripe
CHUNK = 512                               # matmul moving free dim (PSUM bank)

F32 = mybir.dt.float32
F16 = mybir.dt.float16
I32 = mybir.dt.int32
OP = mybir.AluOpType


def build_nc(sb_bufs=5, ps_bufs=8, dve_hwdge=False, n_swq=1):
    nc = bacc.Bacc("TRN2", target_bir_lowering=False, debug=False,
                   num_devices=NCORES, num_swdge_queues=n_swq)
    if dve_hwdge:
        # Register DVE as a third HWDGE engine: its qDVEDynamicHW ring gives
        # the kernel another independent hardware-DGE DMA stream.
        nc.hwdge_engines.add(mybir.EngineType.DVE)
        nc.m.queues.append(mybir.DMAQueue(
            type="dynamic", name="qDVEDynamicHW", blocks=[],
            engine=mybir.EngineType.DVE, location_alt=False,
            num_queues=16, is_HWDGE=True, num_semaphores=0, semaphores=[]))
    x_sh = nc.dram_tensor("x_sh", [SHARD_IH, W], F16, kind="ExternalInput").ap()
    w_d = nc.dram_tensor("w", [KH, KW], F32, kind="ExternalInput").ap()
    b_d = nc.dram_tensor("b", [1], F32, kind="ExternalInput").ap()
    out_sh = nc.dram_tensor("out_sh", [SHARD_OH, OW], F16,
                            kind="ExternalOutput").ap()

    with tile.TileContext(nc) as tc, ExitStack() as ctx:
        consts = ctx.enter_context(tc.tile_pool(name="consts", bufs=1))
        xin = ctx.enter_context(tc.tile_pool(name="xin", bufs=sb_bufs))
        outp = ctx.enter_context(tc.tile_pool(name="outp", bufs=sb_bufs))
        psum = ctx.enter_context(tc.tile_pool(name="psum", bufs=ps_bufs,
                                              space="PSUM"))

        # ---- one-time setup: broadcast weights, build fp16 band matrices ----
        # wb[:, j] = w[j//3, j%3] for all partitions; wb[:, 9] = bias
        wb = consts.tile([128, 10], F32)
        nc.sync.dma_start(wb[:, 0:9], w_d.rearrange("a b -> (a b)")
                          .unsqueeze(0).partition_broadcast(128))
        nc.sync.dma_start(wb[:, 9:10], b_d.unsqueeze(0).partition_broadcast(128))

        # diag[p, m] = p - m ; mask_dy = (diag == dy)
        diag = consts.tile([128, STRIPE_O], I32)
        nc.gpsimd.iota(diag[:], pattern=[[-1, STRIPE_O]], base=0,
                       channel_multiplier=1)
        masks = []
        for dy in range(KH):
            m = consts.tile([128, STRIPE_O], F32, tag=f"mask{dy}")
            nc.vector.tensor_scalar(m[:], diag[:], dy, None, OP.is_equal)
            masks.append(m)
        # band matrices: band16[dx][k, m] = w[k-m, dx] (fp16 stationary)
        bands = []
        bf = consts.tile([128, STRIPE_O], F32)
        for dx in range(KW):
            nc.vector.tensor_scalar(bf[:], masks[0][:], wb[:, dx:dx + 1],
                                    None, OP.mult)
            for dy in range(1, KH):
                j = 3 * dy + dx
                nc.vector.scalar_tensor_tensor(bf[:], masks[dy][:],
                                               wb[:, j:j + 1], bf[:],
                                               OP.mult, OP.add)
            b16 = consts.tile([128, STRIPE_O], F16, tag=f"band{dx}")
            nc.scalar.copy(b16[:], bf[:])
            bands.append(b16)

        # ---- main loop over row stripes ----
        # Every stripe transfer is split into 4 column blocks of 2048 cols
        # (4KB per-partition lines). The block granularity lets matmuls
        # start as soon as the first block lands instead of waiting out a
        # full-stripe DMA (a single 1.9MB HWDGE DMA has ~70us latency).
        nchunks = (OW + CHUNK - 1) // CHUNK            # 16 (last chunk 510)
        BLK = 2048
        nblk = 4

        def in_blocks(s):
            # Inputs ride SWDGE only: they gate the matmuls, and SWDGE both
            # spreads over all 16 engines and keeps latency low (~3us/block).
            r0 = s * STRIPE_O
            n_in = min(STRIPE_I, SHARD_IH - r0)
            xt = xin.tile([n_in, W], F16, name=f"xt{s}", tag="xt")
            for b in range(nblk):
                c0, c1 = b * BLK, (b + 1) * BLK
                gp_dma(xt[:, c0:c1], x_sh[r0:r0 + n_in, c0:c1])
            return xt

        hw_engines = ([nc.sync, nc.scalar, nc.vector] if dve_hwdge
                      else [nc.sync, nc.scalar])
        out_rr = [0]
        swq_rr = [0]

        def gp_dma(out_ap_, in_ap_):
            # Round-robin SWDGE transfers over the allocated qPoolDynamic{i}
            # queues. dma_start always targets queue 0, so the extra queues
            # are reached by emitting the InstDMACopy directly.
            q = swq_rr[0] % n_swq
            swq_rr[0] += 1
            eng = nc.gpsimd
            if q == 0:
                eng.dma_start(out_ap_, in_ap_)
                return
            o, i = bass.balance_dma_aps(out_ap_, in_ap_)
            oap = eng.lower_ap_dma(o, force_symbolic=False,
                                   has_bounds_check=False)
            iap = eng.lower_ap_dma(i, force_symbolic=False,
                                   has_bounds_check=False)
            eng.add_instruction(mybir.InstDMACopy(
                name=nc.get_next_instruction_name(),
                queue=f"qPoolDynamic{q}", mode="Copy",
                ins=[*iap], outs=[*oap], oob_is_err=True,
                cce_op=mybir.AluOpType.bypass,
                bass_cond_hint=None, single_packet=False))

        def out_blocks(s, ot, n_out):
            # Outputs only need to drain by the end: push ~5/8 of them onto
            # the HWDGE queues (round-robin), rest onto SWDGE.
            r0 = s * STRIPE_O
            n_hw = 3 if s % 2 == 0 else 2
            hw = {(s + i) % nblk for i in range(n_hw)}
            for b in range(nblk):
                c0, c1 = b * BLK, min((b + 1) * BLK, OW)
                if b in hw:
                    eng = hw_engines[out_rr[0] % len(hw_engines)]
                    out_rr[0] += 1
                    eng.dma_start(out_sh[r0:r0 + n_out, c0:c1], ot[:, c0:c1])
                else:
                    gp_dma(out_sh[r0:r0 + n_out, c0:c1], ot[:, c0:c1])

        # Out-block emission runs TWO stripes behind compute: their eviction
        # waits are then already satisfied, so they never head-of-line-block
        # the gpsimd FIFO between successive stripes' input triggers.
        pending = []
        for s in range(NSTRIPES):
            r0 = s * STRIPE_O
            n_in = min(STRIPE_I, SHARD_IH - r0)        # 116 (114 for s=8)
            n_out = n_in - (KH - 1)
            xt = in_blocks(s)
            if len(pending) >= 2:
                out_blocks(*pending.pop(0))
            ot = outp.tile([n_out, OW], F16, name=f"ot{s}", tag="ot")
            for c in range(nchunks):
                n0 = c * CHUNK
                free = min(CHUNK, OW - n0)
                pt = psum.tile([n_out, CHUNK], F32)
                for dx in range(KW):
                    nc.tensor.matmul(pt[:, :free],
                                     bands[dx][:n_in, :n_out],
                                     xt[:, n0 + dx:n0 + dx + free],
                                     start=(dx == 0), stop=(dx == KW - 1))
                if c % 2 == 0:
                    nc.scalar.activation(
                        ot[:, n0:n0 + free], pt[:, :free],
                        mybir.ActivationFunctionType.Identity,
                        bias=wb[0:n_out, 9:10])
                else:
                    nc.vector.tensor_scalar(ot[:, n0:n0 + free], pt[:, :free],
                                            wb[0:n_out, 9:10], None, OP.add)
            pending.append((s, ot, n_out))
        while pending:
            out_blocks(*pending.pop(0))
    nc.compile()
    return nc


_nc_cache = {}


def _get_nc(**kw):
    key = tuple(sorted(kw.items()))
    if key not in _nc_cache:
        _nc_cache[key] = build_nc(**kw)
    return _nc_cache[key]


def shard_inputs(x, weight, bias):
    x16 = np.asarray(x, dtype=np.float16)
    weight = np.ascontiguousarray(np.asarray(weight, dtype=np.float32))
    bias = np.ascontiguousarray(np.asarray(bias, dtype=np.float32))
    row0 = [min(c * SHARD_OH, H - SHARD_IH) for c in range(NCORES)]
    in_maps = [{"x_sh": np.ascontiguousarray(x16[r0:r0 + SHARD_IH, :]),
                "w": weight, "b": bias} for r0 in row0]
    return in_maps, row0


def unshard_outputs(results, row0):
    out = np.empty((OH, OW), dtype=np.float32)
    for c in range(NCORES):
        sh = results[c]["out_sh"]
        lo = c * SHARD_OH
        hi = min(lo + SHARD_OH, OH)
        off = lo - row0[c]
        out[lo:hi, :] = sh[off:off + (hi - lo), :].astype(np.float32)
    return out


def kernel(x, weight, bias, **build_kw):
    nc = _get_nc(**build_kw)
    in_maps, row0 = shard_inputs(x, weight, bias)
    res = run_bass_kernel_spmd(nc, in_maps, list(range(NCORES)))
    return unshard_outputs(res.results, row0)


# revision 39
# speedup vs baseline: 1.0615x; 1.0541x over previous
"""TRN2 Bass kernel: 3x3 valid cross-correlation + bias on [8192, 8192] fp32.

Strategy (memory-regime, rel-err budget 2e-2):
- fp16 end-to-end on the wire: the host casts x to fp16, the device computes
  fp16 banded matmuls (1 cycle/row on the PE, same as bf16) accumulating in
  fp32 PSUM, and writes fp16 output that the host casts back to fp32. This
  halves HBM traffic (the roofline for this kernel) and keeps relative error
  ~4e-4, well inside the 2e-2 gate.
- Row sharding across the 8 cores (1026-row input shards incl. the 2-row
  halo; weight/bias replicated). Each core streams 9 stripes of 116 input
  rows (114 output rows) x 8192 cols. Stripe transfers are split into
  2048-col blocks and spread over the SWDGE queue (all 16 SDMA engines,
  ~150 GB/s) plus the two HWDGE queues (~54 GB/s each) to aggregate all
  available dynamic-DMA bandwidth (~195 GB/s effective).
- Per stripe the 3x3 conv is 3 PSUM-accumulated matmuls per 512-col chunk:
  the column (dy) taps become a 3-banded stationary matrix B_dx[k, m] =
  w[k-m, dx] and the row (dx) taps are free-dim shifts of the moving x tile.
  PSUM eviction (+bias, cast to fp16) alternates between the scalar and
  vector engines so neither becomes the critical path.
"""
import numpy as np
from contextlib import ExitStack

import concourse.bass as bass
import concourse.tile as tile
from concourse import mybir, bacc
from concourse.bass_utils import run_bass_kernel_spmd

H = W = 8192
KH = KW = 3
OH, OW = H - KH + 1, W - KW + 1           # 8190 x 8190
NCORES = 8
SHARD_OH = 1024                           # output rows per core
SHARD_IH = SHARD_OH + KH - 1              # 1026 input rows per core
NSTRIPES = 9
STRIPE_O = 114                            # output rows per stripe (8 stripes)
STRIPE_I = STRIPE_O + KH - 1              # 116 input rows per stripe
CHUNK = 512                               # matmul moving free dim (PSUM bank)

F32 = mybir.dt.float32
F16 = mybir.dt.float16
I32 = mybir.dt.int32
OP = mybir.AluOpType


def build_nc(sb_bufs=5, ps_bufs=8, dve_hwdge=False, n_swq=1):
    nc = bacc.Bacc("TRN2", target_bir_lowering=False, debug=False,
                   num_devices=NCORES, num_swdge_queues=n_swq)
    if dve_hwdge:
        # Register DVE as a third HWDGE engine: its qDVEDynamicHW ring gives
        # the kernel another independent hardware-DGE DMA stream.
        nc.hwdge_engines.add(mybir.EngineType.DVE)
        nc.m.queues.append(mybir.DMAQueue(
            type="dynamic", name="qDVEDynamicHW", blocks=[],
            engine=mybir.EngineType.DVE, location_alt=False,
            num_queues=16, is_HWDGE=True, num_semaphores=0, semaphores=[]))
    x_sh = nc.dram_tensor("x_sh", [SHARD_IH, W], F16, kind="ExternalInput").ap()
    w_d = nc.dram_tensor("w", [KH, KW], F32, kind="ExternalInput").ap()
    b_d = nc.dram_tensor("b", [1], F32, kind="ExternalInput").ap()
    out_sh = nc.dram_tensor("out_sh", [SHARD_OH, OW], F16,
                            kind="ExternalOutput").ap()

    with tile.TileContext(nc) as tc, ExitStack() as ctx:
        consts = ctx.enter_context(tc.tile_pool(name="consts", bufs=1))
        xin = ctx.enter_context(tc.tile_pool(name="xin", bufs=sb_bufs))
        outp = ctx.enter_context(tc.tile_pool(name="outp", bufs=sb_bufs))
        psum = ctx.enter_context(tc.tile_pool(name="psum", bufs=ps_bufs,
                                              space="PSUM"))

        # ---- one-time setup: broadcast weights, build fp16 band matrices ----
        # wb[:, j] = w[j//3, j%3] for all partitions; wb[:, 9] = bias
        wb = consts.tile([128, 10], F32)
        nc.sync.dma_start(wb[:, 0:9], w_d.rearrange("a b -> (a b)")
                          .unsqueeze(0).partition_broadcast(128))
        nc.sync.dma_start(wb[:, 9:10], b_d.unsqueeze(0).partition_broadcast(128))

        # diag[p, m] = p - m ; mask_dy = (diag == dy)
        diag = consts.tile([128, STRIPE_O], I32)
        nc.gpsimd.iota(diag[:], pattern=[[-1, STRIPE_O]], base=0,
                       channel_multiplier=1)
        masks = []
        for dy in range(KH):
            m = consts.tile([128, STRIPE_O], F32, tag=f"mask{dy}")
            nc.vector.tensor_scalar(m[:], diag[:], dy, None, OP.is_equal)
            masks.append(m)
        # band matrices: band16[dx][k, m] = w[k-m, dx] (fp16 stationary)
        bands = []
        bf = consts.tile([128, STRIPE_O], F32)
        for dx in range(KW):
            nc.vector.tensor_scalar(bf[:], masks[0][:], wb[:, dx:dx + 1],
                                    None, OP.mult)
            for dy in range(1, KH):
                j = 3 * dy + dx
                nc.vector.scalar_tensor_tensor(bf[:], masks[dy][:],
                                               wb[:, j:j + 1], bf[:],
                                               OP.mult, OP.add)
            b16 = consts.tile([128, STRIPE_O], F16, tag=f"band{dx}")
            nc.scalar.copy(b16[:], bf[:])
            bands.append(b16)

        # ---- main loop over row stripes ----
        # Every stripe transfer is split into 4 column blocks of 2048 cols
        # (4KB per-partition lines). The block granularity lets matmuls
        # start as soon as the first block lands instead of waiting out a
        # full-stripe DMA (a single 1.9MB HWDGE DMA has ~70us latency).
        nchunks = (OW + CHUNK - 1) // CHUNK            # 16 (last chunk 510)
        BLK = 2048
        nblk = 4

        def in_blocks(s):
            # Inputs ride SWDGE only: they gate the matmuls, and SWDGE both
            # spreads over all 16 engines and keeps latency low (~3us/block).
            r0 = s * STRIPE_O
            n_in = min(STRIPE_I, SHARD_IH - r0)
            xt = xin.tile([n_in, W], F16, name=f"xt{s}", tag="xt")
            for b in range(nblk):
                c0, c1 = b * BLK, (b + 1) * BLK
                gp_dma(xt[:, c0:c1], x_sh[r0:r0 + n_in, c0:c1])
            return xt

        hw_engines = ([nc.sync, nc.scalar, nc.vector] if dve_hwdge
                      else [nc.sync, nc.scalar])
        out_rr = [0]
        swq_rr = [0]

        def gp_dma(out_ap_, in_ap_):
            # Round-robin SWDGE transfers over the allocated qPoolDynamic{i}
            # queues. dma_start always targets queue 0, so the extra queues
            # are reached by emitting the InstDMACopy directly.
            q = swq_rr[0] % n_swq
            swq_rr[0] += 1
            eng = nc.gpsimd
            if q == 0:
                eng.dma_start(out_ap_, in_ap_)
                return
            o, i = bass.balance_dma_aps(out_ap_, in_ap_)
            oap = eng.lower_ap_dma(o, force_symbolic=False,
                                   has_bounds_check=False)
            iap = eng.lower_ap_dma(i, force_symbolic=False,
                                   has_bounds_check=False)
            eng.add_instruction(mybir.InstDMACopy(
                name=nc.get_next_instruction_name(),
                queue=f"qPoolDynamic{q}", mode="Copy",
                ins=[*iap], outs=[*oap], oob_is_err=True,
                cce_op=mybir.AluOpType.bypass,
                bass_cond_hint=None, single_packet=False))

        def out_blocks(s, ot, n_out):
            # Outputs only need to drain by the end: push ~5/8 of them onto
            # the HWDGE queues (round-robin), rest onto SWDGE. The last two
            # stripes drain after compute finishes, so they go SWDGE-heavy
            # (~150 GB/s) to shorten that tail instead of ~65 GB/s HWDGE.
            r0 = s * STRIPE_O
            n_hw = 1 if s >= NSTRIPES - 2 else (3 if s % 2 == 0 else 2)
            hw = {(s + i) % nblk for i in range(n_hw)}
            for b in range(nblk):
                c0, c1 = b * BLK, min((b + 1) * BLK, OW)
                if b in hw:
                    eng = hw_engines[out_rr[0] % len(hw_engines)]
                    out_rr[0] += 1
                    eng.dma_start(out_sh[r0:r0 + n_out, c0:c1], ot[:, c0:c1])
                else:
                    gp_dma(out_sh[r0:r0 + n_out, c0:c1], ot[:, c0:c1])

        # Out-block emission runs TWO stripes behind compute: their eviction
        # waits are then already satisfied, so they never head-of-line-block
        # the gpsimd FIFO between successive stripes' input triggers.
        pending = []
        for s in range(NSTRIPES):
            r0 = s * STRIPE_O
            n_in = min(STRIPE_I, SHARD_IH - r0)        # 116 (114 for s=8)
            n_out = n_in - (KH - 1)
            xt = in_blocks(s)
            if len(pending) >= 2:
                out_blocks(*pending.pop(0))
            ot = outp.tile([n_out, OW], F16, name=f"ot{s}", tag="ot")
            for c in range(nchunks):
                n0 = c * CHUNK
                free = min(CHUNK, OW - n0)
                pt = psum.tile([n_out, CHUNK], F32)
                for dx in range(KW):
                    nc.tensor.matmul(pt[:, :free],
                                     bands[dx][:n_in, :n_out],
                                     xt[:, n0 + dx:n0 + dx + free],
                                     start=(dx == 0), stop=(dx == KW - 1))
                if c % 2 == 0:
                    nc.scalar.activation(
                        ot[:, n0:n0 + free], pt[:, :free],
                        mybir.ActivationFunctionType.Identity,
                        bias=wb[0:n_out, 9:10])
                else:
                    nc.vector.tensor_scalar(ot[:, n0:n0 + free], pt[:, :free],
                                            wb[0:n_out, 9:10], None, OP.add)
            pending.append((s, ot, n_out))
        while pending:
            out_blocks(*pending.pop(0))
    nc.compile()
    return nc


_nc_cache = {}


def _get_nc(**kw):
    key = tuple(sorted(kw.items()))
    if key not in _nc_cache:
        _nc_cache[key] = build_nc(**kw)
    return _nc_cache[key]


def shard_inputs(x, weight, bias):
    x16 = np.asarray(x, dtype=np.float16)
    weight = np.ascontiguousarray(np.asarray(weight, dtype=np.float32))
    bias = np.ascontiguousarray(np.asarray(bias, dtype=np.float32))
    row0 = [min(c * SHARD_OH, H - SHARD_IH) for c in range(NCORES)]
    in_maps = [{"x_sh": np.ascontiguousarray(x16[r0:r0 + SHARD_IH, :]),
                "w": weight, "b": bias} for r0 in row0]
    return in_maps, row0


def unshard_outputs(results, row0):
    out = np.empty((OH, OW), dtype=np.float32)
    for c in range(NCORES):
        sh = results[c]["out_sh"]
        lo = c * SHARD_OH
        hi = min(lo + SHARD_OH, OH)
        off = lo - row0[c]
        out[lo:hi, :] = sh[off:off + (hi - lo), :].astype(np.float32)
    return out


def kernel(x, weight, bias, **build_kw):
    nc = _get_nc(**build_kw)
    in_maps, row0 = shard_inputs(x, weight, bias)
    res = run_bass_kernel_spmd(nc, in_maps, list(range(NCORES)))
    return unshard_outputs(res.results, row0)
